# revision 15
# baseline (speedup 1.0000x reference)
"""Trainium2 Bass kernel for nn_MemoryModule (free-energy memory module).

Computation (per batch row b):
    h      = gelu_exact(key @ W1.T + b1)             # (B, 64)
    pred   = h @ W2.T + b2                           # (B, 512)
    prec   = softplus(key @ prec_W.T + prec_b) + .01 # (B, 512)
    error  = clip(value - pred, -3, 3)               # (B, 512)
    F      = mean_v(prec * error^2 - ln(prec))       # (B,)
Returns (F, pred, prec, error).

Strategy: pure batch data-parallelism over 8 NeuronCores (8192 rows each).
Host pre-transposes each core's key shard to keyT [512, 8192] (bf16) so the
d-contraction needs no on-chip transpose; keyT stays fully resident in SBUF.
All matmuls run in bf16 (fp32 matmuls lower as two PE passes on trn2).

Two compute phases per core, separated by a scheduler fence so the ACT
engine's activation-table set is switched exactly once:
  Phase A (gelu table):  h^T strips = Gelu(W1T.T @ keyT + b1), kept resident.
  Phase B (exp/ln table): per 128-row sub-tile: prec/pred matmuls,
  softplus = ln(1+exp(u)), prec, ln(prec) (with fused free-dim sum via
  activation accum_out), error, and sum(prec*err^2) via the fused
  scalar_tensor_tensor accumulate. DMA is batched at 1 MiB granularity
  ([128, 2048] tiles spanning 4 sub-tiles) for full HBM bandwidth.
F is assembled from the two per-row sums and stored via a PE transpose.
"""

import os
import sys

sys.path.insert(0, "/opt/trn_rl_repo")

import numpy as np
import ml_dtypes
from contextlib import ExitStack

import concourse.bass as bass
import concourse.bacc as bacc
import concourse.tile as tile
import concourse.masks as masks
import concourse.hw_specs as hw_specs
from concourse import mybir
from concourse.bass_utils import run_bass_kernel_spmd

N_CORES = 8
B_FULL = 65536
BC = B_FULL // N_CORES  # 8192 rows per core
DK = 512
DV = 512
DH = 64

F32 = mybir.dt.float32
BF16 = mybir.dt.bfloat16
AF = mybir.ActivationFunctionType
OP = mybir.AluOpType

_CACHE = {}
LAST_RESULTS = None  # BassKernelResults of the most recent run (for test harness)


def _doctor_act_tables(arch: str) -> None:
    """Restrict Exp/Ln to the combined natural_log_exp_and_others set so the
    table-load pass never thrashes between exp_and_others and natural_log."""
    tables = hw_specs.get_activation_tables(arch)
    for name, funcs in tables.items():
        if name != "natural_log_exp_and_others":
            funcs.discard(AF.Exp)
            funcs.discard(AF.Ln)


def build_nc(bc: int = BC):
    """Build + compile the Bass module for one core processing `bc` rows."""
    assert bc % 2048 == 0
    n_tiles = bc // 512   # 512-row tiles (phase B outer granularity)
    n_sub = bc // 128     # 128-row sub-tiles
    n_grp = bc // 2048    # keyT big-tile groups

    nc = bacc.Bacc("TRN2", target_bir_lowering=False, debug=False,
                   num_devices=N_CORES)

    keyT = nc.dram_tensor("keyT", [DK, bc], BF16, kind="ExternalInput").ap()
    value = nc.dram_tensor("value", [bc, DV], BF16, kind="ExternalInput").ap()
    w1t_d = nc.dram_tensor("w1t", [DK, DH], BF16, kind="ExternalInput").ap()
    b1_d = nc.dram_tensor("b1", [DH, 1], F32, kind="ExternalInput").ap()
    w2ta_d = nc.dram_tensor("w2ta", [DH + 1, DV], BF16, kind="ExternalInput").ap()
    pwt_d = nc.dram_tensor("pwt", [DK, DV], BF16, kind="ExternalInput").ap()
    pb_d = nc.dram_tensor("pb", [1, DV], BF16, kind="ExternalInput").ap()

    pred_d = nc.dram_tensor("pred", [bc, DV], BF16, kind="ExternalOutput").ap()
    prec_d = nc.dram_tensor("prec", [bc, DV], BF16, kind="ExternalOutput").ap()
    err_d = nc.dram_tensor("err", [bc, DV], BF16, kind="ExternalOutput").ap()
    f_d = nc.dram_tensor("F", [bc], F32, kind="ExternalOutput").ap()

    # [t, p, s, v] views: 512-tile t, partition p, sub-tile s, feature v
    value_r = value.rearrange("(t s p) v -> t p s v", s=4, p=128)
    pred_r = pred_d.rearrange("(t s p) v -> t p s v", s=4, p=128)
    prec_r = prec_d.rearrange("(t s p) v -> t p s v", s=4, p=128)
    err_r = err_d.rearrange("(t s p) v -> t p s v", s=4, p=128)

    with tile.TileContext(nc) as tc, ExitStack() as ctx:
        consts = ctx.enter_context(tc.tile_pool(name="consts", bufs=1))
        ktp = ctx.enter_context(tc.tile_pool(name="kt", bufs=1))
        htp = ctx.enter_context(tc.tile_pool(name="ht", bufs=1))
        strips = ctx.enter_context(tc.tile_pool(name="strips", bufs=1))
        vp = ctx.enter_context(tc.tile_pool(name="val", bufs=2))
        obig = ctx.enter_context(tc.tile_pool(name="obig", bufs=2))
        ep = ctx.enter_context(tc.tile_pool(name="expt", bufs=2))
        spp = ctx.enter_context(tc.tile_pool(name="sp", bufs=2))
        mp = ctx.enter_context(tc.tile_pool(name="m", bufs=2))
        e0p = ctx.enter_context(tc.tile_pool(name="e0", bufs=2))
        scrp = ctx.enter_context(tc.tile_pool(name="scr", bufs=2))
        ps_h = ctx.enter_context(tc.tile_pool(name="ps_h", bufs=1, space="PSUM"))
        ps_prec = ctx.enter_context(tc.tile_pool(name="ps_prec", bufs=2, space="PSUM"))
        ps_pred = ctx.enter_context(tc.tile_pool(name="ps_pred", bufs=2, space="PSUM"))

        # ---- constants ----
        w1t = [consts.tile([128, DH], BF16, tag=f"w1t{c}", name=f"w1t{c}")
               for c in range(4)]
        for c in range(4):
            nc.sync.dma_start(w1t[c][:], w1t_d[c * 128:(c + 1) * 128, :])
        b1 = consts.tile([DH, 1], F32, tag="b1")
        nc.sync.dma_start(b1[:], b1_d[:])
        w2ta = consts.tile([DH + 1, DV], BF16, tag="w2ta")
        nc.sync.dma_start(w2ta[:], w2ta_d[:])
        pwt = [consts.tile([128, DV], BF16, tag=f"pwt{c}", name=f"pwt{c}")
               for c in range(4)]
        for c in range(4):
            nc.sync.dma_start(pwt[c][:], pwt_d[c * 128:(c + 1) * 128, :])
        pb = consts.tile([1, DV], BF16, tag="pb")
        nc.sync.dma_start(pb[:], pb_d[:])
        ones = consts.tile([1, 128], BF16, tag="ones")
        nc.vector.memset(ones[:], 1.0)
        ident = consts.tile([128, 128], F32, tag="ident")
        masks.make_identity(nc, ident[:])
        c001 = consts.tile([128, 1], F32, tag="c001")
        nc.vector.memset(c001[:], 0.01)

        sumlog = strips.tile([128, n_sub], F32, tag="sumlog")
        sum1 = strips.tile([128, n_sub], F32, tag="sum1")
        fstrip = strips.tile([128, n_sub], F32, tag="fstrip")

        # resident keyT big tiles: kt[c][g] = [128, 2048] bf16
        kt = [[ktp.tile([128, 2048], BF16, tag=f"kt{c}_{g}", name=f"kt{c}_{g}")
               for g in range(n_grp)] for c in range(4)]
        for c in range(4):
            for g in range(n_grp):
                eng = nc.sync if (c + g) % 2 == 0 else nc.scalar
                eng.dma_start(
                    kt[c][g][:],
                    keyT[c * 128:(c + 1) * 128, g * 2048:(g + 1) * 2048])

        ht = []

        # ---- phase A: h^T = gelu(W1T.T @ keyT + b1), resident bf16 ----
        for t in range(n_tiles):
            g, off = divmod(t, 4)
            ph = ps_h.tile([DH, 512], F32)
            for c in range(4):
                nc.tensor.matmul(ph[:], w1t[c][:],
                                 kt[c][g][:, off * 512:(off + 1) * 512],
                                 start=(c == 0), stop=(c == 3))
            h = htp.tile([DH + 1, 512], BF16, tag=f"ht{t}", name=f"ht{t}")
            nc.vector.memset(h[DH:DH + 1, :], 1.0)
            nc.scalar.activation(h[0:DH, :], ph[:], AF.Gelu, bias=b1[:, 0:1])
            ht.append(h)

        # single act-table switch past this point
        tc.no_sync_barrier()

        # ---- phase B: per 512-tile (4 sub-tiles), 1MiB-batched DMA ----
        for t in range(n_tiles):
            g, off = divmod(t, 4)
            val = vp.tile([128, 2048], BF16, tag="val")
            nc.sync.dma_start(val[:].rearrange("p (s v) -> p s v", s=4),
                              value_r[t])
            pred_b = obig.tile([128, 2048], BF16, tag="pred_b")
            prec_b = obig.tile([128, 2048], BF16, tag="prec_b")
            err_b = obig.tile([128, 2048], BF16, tag="err_b")
            ex_big = ep.tile([128, 2048], F32, tag="ex_big")
            sp_big = spp.tile([128, 2048], F32, tag="sp_big")

            for half in range(2):
                pp = ps_prec.tile([128, 2 * DV], F32)
                for j in range(2):
                    s = half * 2 + j
                    bo = off * 512 + s * 128
                    jsl = slice(j * 512, (j + 1) * 512)
                    nc.tensor.matmul(pp[:, jsl], ones[:], pb[:],
                                     start=True, stop=False)
                    for c in range(4):
                        nc.tensor.matmul(pp[:, jsl], kt[c][g][:, bo:bo + 128],
                                         pwt[c][:], start=False, stop=(c == 3))
                nc.scalar.activation(
                    ex_big[:, half * 1024:(half + 1) * 1024], pp[:], AF.Exp)

            for s in range(4):
                i = t * 4 + s
                sl = slice(s * 512, (s + 1) * 512)
                pq = ps_pred.tile([128, DV], F32)
                nc.tensor.matmul(pq[:], ht[t][:, s * 128:(s + 1) * 128],
                                 w2ta[:], start=True, stop=True)
                # pred copy: alternate ACT/DVE to balance engine load
                if s % 2 == 0:
                    nc.scalar.activation(pred_b[:, sl], pq[:], AF.Copy)
                else:
                    nc.vector.tensor_copy(pred_b[:, sl], pq[:])

            # one wide softplus ln for the whole 512-tile
            nc.scalar.activation(sp_big[:], ex_big[:], AF.Ln, bias=1.0)

            for s in range(4):
                i = t * 4 + s
                sl = slice(s * 512, (s + 1) * 512)
                nc.vector.tensor_scalar_add(prec_b[:, sl], sp_big[:, sl], 0.01)
                lnscr = scrp.tile([128, DV], F32, tag="lnscr")
                nc.scalar.activation(lnscr[:], sp_big[:, sl], AF.Ln,
                                     bias=c001[:, 0:1],
                                     accum_out=sumlog[:, i:i + 1])

                e0 = e0p.tile([128, DV], BF16, tag="e0")
                nc.vector.tensor_sub(e0[:], val[:, sl], pred_b[:, sl])
                nc.vector.tensor_scalar(err_b[:, sl], e0[:], 3.0, -3.0,
                                        OP.min, OP.max)

                m = mp.tile([128, DV], BF16, tag="m")
                nc.vector.tensor_mul(m[:], prec_b[:, sl], err_b[:, sl])
                stt_out = scrp.tile([128, DV], BF16, tag="stt")
                nc.vector.scalar_tensor_tensor(
                    out=stt_out[:], in0=m[:], scalar=1.0 / DV, in1=err_b[:, sl],
                    op0=OP.mult, op1=OP.mult,
                    accum_out=sum1[:, i:i + 1])

            nc.sync.dma_start(pred_r[t], pred_b[:].rearrange("p (s v) -> p s v", s=4))
            nc.sync.dma_start(prec_r[t], prec_b[:].rearrange("p (s v) -> p s v", s=4))
            nc.sync.dma_start(err_r[t], err_b[:].rearrange("p (s v) -> p s v", s=4))

        # ---- finale: F = sum1 - sumlog/DV, transpose, store ----
        nc.vector.scalar_tensor_tensor(
            out=fstrip[:], in0=sumlog[:], scalar=-1.0 / DV, in1=sum1[:],
            op0=OP.mult, op1=OP.add)
        pf = ps_h.tile([n_sub, 128], F32, tag="psf", bufs=1)
        nc.tensor.transpose(pf[:], fstrip[:], ident[:])
        fout = consts.tile([n_sub, 128], F32, tag="fout")
        nc.scalar.activation(fout[:], pf[:], AF.Copy)
        nc.sync.dma_start(f_d.rearrange("(t p) -> t p", p=128)[:], fout[:])

    _doctor_act_tables(nc.m.arch)
    nc.compile()
    return nc


def _install_prof_hook():
    """Make trace=True work under axon (antenv.axon_hooks is absent here)."""
    import contextlib
    import ctypes
    import types

    if "antenv.axon_hooks" in sys.modules:
        return
    so_path = "/opt/axon/libaxon_pjrt.so"
    try:
        lib = ctypes.CDLL(so_path)
    except OSError:
        return
    if not hasattr(lib, "axon_start_nrt_profile"):
        return
    lib.axon_start_nrt_profile.argtypes = [ctypes.POINTER(ctypes.c_int64),
                                           ctypes.c_size_t]
    lib.axon_start_nrt_profile.restype = ctypes.c_int64
    lib.axon_stop_nrt_profile.argtypes = [ctypes.c_char_p]
    lib.axon_stop_nrt_profile.restype = ctypes.c_int64

    @contextlib.contextmanager
    def _hook(output_dir, device_ids):
        import jax

        jax.devices()
        if device_ids:
            ids = (ctypes.c_int64 * len(device_ids))(*device_ids)
            rc = lib.axon_start_nrt_profile(ids, len(device_ids))
        else:
            rc = lib.axon_start_nrt_profile(None, 0)
        if rc != 0:
            raise RuntimeError(f"axon_start_nrt_profile rc={rc}")
        try:
            yield
        finally:
            n = lib.axon_stop_nrt_profile(str(output_dir).encode())
            print(f"ntff profile: {n} file(s) in {output_dir}", file=sys.stderr)

    mod = types.ModuleType("antenv.axon_hooks")
    mod.get_axon_ntff_profile_hook = lambda: _hook
    mod.set_axon_ntff_profile_hook = lambda h: None
    sys.modules["antenv.axon_hooks"] = mod

    from concourse import bass_utils

    bass_utils.upload_artifacts = lambda tmpdir: f"local:{tmpdir}"


def kernel(key, value, mem_W1, mem_b1, mem_W2, mem_b2, prec_W, prec_b,
           _trace=False, _tmpdir=None):
    global LAST_RESULTS
    key = np.asarray(key, dtype=np.float32)
    value = np.asarray(value, dtype=np.float32)
    mem_W1 = np.asarray(mem_W1, dtype=np.float32)
    mem_b1 = np.asarray(mem_b1, dtype=np.float32)
    mem_W2 = np.asarray(mem_W2, dtype=np.float32)
    mem_b2 = np.asarray(mem_b2, dtype=np.float32)
    prec_W = np.asarray(prec_W, dtype=np.float32)
    prec_b = np.asarray(prec_b, dtype=np.float32)

    if "nc" not in _CACHE:
        _CACHE["nc"] = build_nc(BC)
    nc = _CACHE["nc"]

    bf = ml_dtypes.bfloat16
    w1t = np.ascontiguousarray(mem_W1.T).astype(bf)          # [512, 64]
    b1 = np.ascontiguousarray(mem_b1[:, None])               # [64, 1]
    w2ta = np.ascontiguousarray(
        np.vstack([mem_W2.T, mem_b2[None, :]])).astype(bf)   # [65, 512]
    pwt = np.ascontiguousarray(prec_W.T).astype(bf)          # [512, 512]
    pb = np.ascontiguousarray(prec_b[None, :]).astype(bf)    # [1, 512]

    in_maps = []
    for c in range(N_CORES):
        sl = slice(c * BC, (c + 1) * BC)
        in_maps.append({
            "keyT": np.ascontiguousarray(key[sl].T).astype(bf),
            "value": np.ascontiguousarray(value[sl]).astype(bf),
            "w1t": w1t, "b1": b1, "w2ta": w2ta, "pwt": pwt, "pb": pb,
        })

    if _trace or os.environ.get("BASS_TRACE"):
        _install_prof_hook()
    res = run_bass_kernel_spmd(nc, in_maps, list(range(N_CORES)),
                               trace=_trace, tmpdir=_tmpdir)
    LAST_RESULTS = res

    F = np.concatenate([res.results[c]["F"] for c in range(N_CORES)], axis=0)
    pred = np.concatenate([res.results[c]["pred"] for c in range(N_CORES)], axis=0)
    prec = np.concatenate([res.results[c]["prec"] for c in range(N_CORES)], axis=0)
    err = np.concatenate([res.results[c]["err"] for c in range(N_CORES)], axis=0)
    return (F.astype(np.float32), pred.astype(np.float32),
            prec.astype(np.float32), err.astype(np.float32))


# revision 16
# speedup vs baseline: 1.1361x; 1.1361x over previous
"""Trainium2 Bass kernel for nn_MemoryModule (free-energy memory module).

Computation (per batch row b):
    h      = gelu_exact(key @ W1.T + b1)             # (B, 64)
    pred   = h @ W2.T + b2                           # (B, 512)
    prec   = softplus(key @ prec_W.T + prec_b) + .01 # (B, 512)
    error  = clip(value - pred, -3, 3)               # (B, 512)
    F      = mean_v(prec * error^2 - ln(prec))       # (B,)
Returns (F, pred, prec, error).

Strategy: pure batch data-parallelism over 8 NeuronCores (8192 rows each).
Host pre-transposes each core's key shard to keyT [512, 8192] (bf16) so the
d-contraction needs no on-chip transpose; keyT stays fully resident in SBUF.
All matmuls run in bf16 (fp32 matmuls lower as two PE passes on trn2).

Two compute phases per core, separated by a scheduler fence so the ACT
engine's activation-table set is switched exactly once:
  Phase A (gelu table):  h^T strips = Gelu(W1T.T @ keyT + b1), kept resident.
  Phase B (exp/ln table): per 128-row sub-tile: prec/pred matmuls,
  softplus = ln(1+exp(u)), prec, ln(prec) (with fused free-dim sum via
  activation accum_out), error, and sum(prec*err^2) via the fused
  scalar_tensor_tensor accumulate. DMA is batched at 1 MiB granularity
  ([128, 2048] tiles spanning 4 sub-tiles) for full HBM bandwidth.
F is assembled from the two per-row sums and stored via a PE transpose.
"""

import os
import sys

sys.path.insert(0, "/opt/trn_rl_repo")

import numpy as np
import ml_dtypes
from contextlib import ExitStack

import concourse.bass as bass
import concourse.bacc as bacc
import concourse.tile as tile
import concourse.masks as masks
import concourse.hw_specs as hw_specs
from concourse import mybir
from concourse.bass_utils import run_bass_kernel_spmd

N_CORES = 8
B_FULL = 65536
BC = B_FULL // N_CORES  # 8192 rows per core
DK = 512
DV = 512
DH = 64

F32 = mybir.dt.float32
BF16 = mybir.dt.bfloat16
AF = mybir.ActivationFunctionType
OP = mybir.AluOpType

_CACHE = {}
LAST_RESULTS = None  # BassKernelResults of the most recent run (for test harness)


def _doctor_act_tables(arch: str) -> None:
    """Restrict Exp/Ln to the combined natural_log_exp_and_others set so the
    table-load pass never thrashes between exp_and_others and natural_log."""
    tables = hw_specs.get_activation_tables(arch)
    for name, funcs in tables.items():
        if name != "natural_log_exp_and_others":
            funcs.discard(AF.Exp)
            funcs.discard(AF.Ln)


def build_nc(bc: int = BC):
    """Build + compile the Bass module for one core processing `bc` rows."""
    assert bc % 2048 == 0
    n_tiles = bc // 512   # 512-row tiles (phase B outer granularity)
    n_sub = bc // 128     # 128-row sub-tiles
    n_grp = bc // 2048    # keyT big-tile groups

    nc = bacc.Bacc("TRN2", target_bir_lowering=False, debug=False,
                   num_devices=N_CORES)

    keyT = nc.dram_tensor("keyT", [DK, bc], BF16, kind="ExternalInput").ap()
    value = nc.dram_tensor("value", [bc, DV], BF16, kind="ExternalInput").ap()
    w1t_d = nc.dram_tensor("w1t", [DK, DH], BF16, kind="ExternalInput").ap()
    b1_d = nc.dram_tensor("b1", [DH, 1], F32, kind="ExternalInput").ap()
    w2ta_d = nc.dram_tensor("w2ta", [DH + 1, DV], BF16, kind="ExternalInput").ap()
    pwt_d = nc.dram_tensor("pwt", [DK, DV], BF16, kind="ExternalInput").ap()
    pb_d = nc.dram_tensor("pb", [1, DV], BF16, kind="ExternalInput").ap()

    pred_d = nc.dram_tensor("pred", [bc, DV], BF16, kind="ExternalOutput").ap()
    prec_d = nc.dram_tensor("prec", [bc, DV], BF16, kind="ExternalOutput").ap()
    err_d = nc.dram_tensor("err", [bc, DV], BF16, kind="ExternalOutput").ap()
    f_d = nc.dram_tensor("F", [bc], F32, kind="ExternalOutput").ap()

    # [t, p, s, v] views: 512-tile t, partition p, sub-tile s, feature v
    value_r = value.rearrange("(t s p) v -> t p s v", s=4, p=128)
    pred_r = pred_d.rearrange("(t s p) v -> t p s v", s=4, p=128)
    prec_r = prec_d.rearrange("(t s p) v -> t p s v", s=4, p=128)
    err_r = err_d.rearrange("(t s p) v -> t p s v", s=4, p=128)

    with tile.TileContext(nc) as tc, ExitStack() as ctx:
        consts = ctx.enter_context(tc.tile_pool(name="consts", bufs=1))
        ktp = ctx.enter_context(tc.tile_pool(name="kt", bufs=1))
        htp = ctx.enter_context(tc.tile_pool(name="ht", bufs=1))
        strips = ctx.enter_context(tc.tile_pool(name="strips", bufs=1))
        vp = ctx.enter_context(tc.tile_pool(name="val", bufs=2))
        obig = ctx.enter_context(tc.tile_pool(name="obig", bufs=2))
        ep = ctx.enter_context(tc.tile_pool(name="expt", bufs=2))
        spp = ctx.enter_context(tc.tile_pool(name="sp", bufs=2))
        mp = ctx.enter_context(tc.tile_pool(name="m", bufs=2))
        e0p = ctx.enter_context(tc.tile_pool(name="e0", bufs=2))
        scrp = ctx.enter_context(tc.tile_pool(name="scr", bufs=2))
        ps_h = ctx.enter_context(tc.tile_pool(name="ps_h", bufs=2, space="PSUM"))
        ps_prec = ctx.enter_context(tc.tile_pool(name="ps_prec", bufs=3, space="PSUM"))
        ps_pred = ctx.enter_context(tc.tile_pool(name="ps_pred", bufs=2, space="PSUM"))

        # ---- constants ----
        w1t = [consts.tile([128, DH], BF16, tag=f"w1t{c}", name=f"w1t{c}")
               for c in range(4)]
        for c in range(4):
            nc.sync.dma_start(w1t[c][:], w1t_d[c * 128:(c + 1) * 128, :])
        b1 = consts.tile([DH, 1], F32, tag="b1")
        nc.sync.dma_start(b1[:], b1_d[:])
        w2ta = consts.tile([DH + 1, DV], BF16, tag="w2ta")
        nc.sync.dma_start(w2ta[:], w2ta_d[:])
        pwt = [consts.tile([128, DV], BF16, tag=f"pwt{c}", name=f"pwt{c}")
               for c in range(4)]
        for c in range(4):
            nc.sync.dma_start(pwt[c][:], pwt_d[c * 128:(c + 1) * 128, :])
        pb = consts.tile([1, DV], BF16, tag="pb")
        nc.sync.dma_start(pb[:], pb_d[:])
        ones = consts.tile([1, 128], BF16, tag="ones")
        nc.vector.memset(ones[:], 1.0)
        ident = consts.tile([128, 128], F32, tag="ident")
        masks.make_identity(nc, ident[:])
        c001 = consts.tile([128, 1], F32, tag="c001")
        nc.vector.memset(c001[:], 0.01)

        sumlog = strips.tile([128, n_sub], F32, tag="sumlog")
        sum1 = strips.tile([128, n_sub], F32, tag="sum1")
        fstrip = strips.tile([128, n_sub], F32, tag="fstrip")

        # resident keyT big tiles: kt[c][g] = [128, 2048] bf16
        kt = [[ktp.tile([128, 2048], BF16, tag=f"kt{c}_{g}", name=f"kt{c}_{g}")
               for g in range(n_grp)] for c in range(4)]
        for c in range(4):
            for g in range(n_grp):
                nc.sync.dma_start(
                    kt[c][g][:],
                    keyT[c * 128:(c + 1) * 128, g * 2048:(g + 1) * 2048])

        ht = []

        # ---- phase A: h^T = gelu(W1T.T @ keyT + b1), resident bf16 ----
        for t in range(n_tiles):
            g, off = divmod(t, 4)
            ph = ps_h.tile([DH, 512], F32)
            for c in range(4):
                nc.tensor.matmul(ph[:], w1t[c][:],
                                 kt[c][g][:, off * 512:(off + 1) * 512],
                                 start=(c == 0), stop=(c == 3))
            h = htp.tile([DH + 1, 512], BF16, tag=f"ht{t}", name=f"ht{t}")
            nc.vector.memset(h[DH:DH + 1, :], 1.0)
            nc.scalar.activation(h[0:DH, :], ph[:], AF.Gelu, bias=b1[:, 0:1])
            ht.append(h)

        # single act-table switch past this point
        tc.no_sync_barrier()

        # ---- phase B: per 512-tile (4 sub-tiles), 1MiB-batched DMA ----
        for t in range(n_tiles):
            g, off = divmod(t, 4)
            val = vp.tile([128, 2048], BF16, tag="val")
            nc.sync.dma_start(val[:].rearrange("p (s v) -> p s v", s=4),
                              value_r[t])
            pred_b = obig.tile([128, 2048], BF16, tag="pred_b")
            prec_b = obig.tile([128, 2048], BF16, tag="prec_b")
            err_b = obig.tile([128, 2048], BF16, tag="err_b")
            ex_big = ep.tile([128, 2048], F32, tag="ex_big")
            sp_big = spp.tile([128, 2048], F32, tag="sp_big")

            for s in range(4):
                i = t * 4 + s
                bo = off * 512 + s * 128  # batch offset inside kt big tile
                sl = slice(s * 512, (s + 1) * 512)

                pp = ps_prec.tile([128, DV], F32)
                nc.tensor.matmul(pp[:], ones[:], pb[:], start=True, stop=False)
                for c in range(4):
                    nc.tensor.matmul(pp[:], kt[c][g][:, bo:bo + 128],
                                     pwt[c][:], start=False, stop=(c == 3))

                pq = ps_pred.tile([128, DV], F32)
                nc.tensor.matmul(pq[:], ht[t][:, s * 128:(s + 1) * 128],
                                 w2ta[:], start=True, stop=True)

                nc.scalar.activation(ex_big[:, sl], pp[:], AF.Exp)
                # pred copy: alternate ACT/DVE to balance engine load
                if s % 2 == 0:
                    nc.scalar.activation(pred_b[:, sl], pq[:], AF.Copy)
                else:
                    nc.vector.tensor_copy(pred_b[:, sl], pq[:])

            # one wide softplus ln for the whole 512-tile
            nc.scalar.activation(sp_big[:], ex_big[:], AF.Ln, bias=1.0)

            for s in range(4):
                i = t * 4 + s
                sl = slice(s * 512, (s + 1) * 512)
                nc.vector.tensor_scalar_add(prec_b[:, sl], sp_big[:, sl], 0.01)
                lnscr = scrp.tile([128, DV], F32, tag="lnscr")
                nc.scalar.activation(lnscr[:], sp_big[:, sl], AF.Ln,
                                     bias=c001[:, 0:1],
                                     accum_out=sumlog[:, i:i + 1])

                e0 = e0p.tile([128, DV], BF16, tag="e0")
                nc.vector.tensor_sub(e0[:], val[:, sl], pred_b[:, sl])
                nc.vector.tensor_scalar(err_b[:, sl], e0[:], 3.0, -3.0,
                                        OP.min, OP.max)

                m = mp.tile([128, DV], BF16, tag="m")
                nc.vector.tensor_mul(m[:], prec_b[:, sl], err_b[:, sl])
                stt_out = scrp.tile([128, DV], BF16, tag="stt")
                nc.vector.scalar_tensor_tensor(
                    out=stt_out[:], in0=m[:], scalar=1.0 / DV, in1=err_b[:, sl],
                    op0=OP.mult, op1=OP.mult,
                    accum_out=sum1[:, i:i + 1])

            nc.sync.dma_start(pred_r[t], pred_b[:].rearrange("p (s v) -> p s v", s=4))
            nc.sync.dma_start(prec_r[t], prec_b[:].rearrange("p (s v) -> p s v", s=4))
            nc.sync.dma_start(err_r[t], err_b[:].rearrange("p (s v) -> p s v", s=4))

        # ---- finale: F = sum1 - sumlog/DV, transpose, store ----
        nc.vector.scalar_tensor_tensor(
            out=fstrip[:], in0=sumlog[:], scalar=-1.0 / DV, in1=sum1[:],
            op0=OP.mult, op1=OP.add)
        pf = ps_h.tile([n_sub, 128], F32, tag="psf", bufs=1)
        nc.tensor.transpose(pf[:], fstrip[:], ident[:])
        fout = consts.tile([n_sub, 128], F32, tag="fout")
        nc.scalar.activation(fout[:], pf[:], AF.Copy)
        nc.sync.dma_start(f_d.rearrange("(t p) -> t p", p=128)[:], fout[:])

    _doctor_act_tables(nc.m.arch)
    nc.compile()
    return nc


def _install_prof_hook():
    """Make trace=True work under axon (antenv.axon_hooks is absent here)."""
    import contextlib
    import ctypes
    import types

    if "antenv.axon_hooks" in sys.modules:
        return
    so_path = "/opt/axon/libaxon_pjrt.so"
    try:
        lib = ctypes.CDLL(so_path)
    except OSError:
        return
    if not hasattr(lib, "axon_start_nrt_profile"):
        return
    lib.axon_start_nrt_profile.argtypes = [ctypes.POINTER(ctypes.c_int64),
                                           ctypes.c_size_t]
    lib.axon_start_nrt_profile.restype = ctypes.c_int64
    lib.axon_stop_nrt_profile.argtypes = [ctypes.c_char_p]
    lib.axon_stop_nrt_profile.restype = ctypes.c_int64

    @contextlib.contextmanager
    def _hook(output_dir, device_ids):
        import jax

        jax.devices()
        if device_ids:
            ids = (ctypes.c_int64 * len(device_ids))(*device_ids)
            rc = lib.axon_start_nrt_profile(ids, len(device_ids))
        else:
            rc = lib.axon_start_nrt_profile(None, 0)
        if rc != 0:
            raise RuntimeError(f"axon_start_nrt_profile rc={rc}")
        try:
            yield
        finally:
            n = lib.axon_stop_nrt_profile(str(output_dir).encode())
            print(f"ntff profile: {n} file(s) in {output_dir}", file=sys.stderr)

    mod = types.ModuleType("antenv.axon_hooks")
    mod.get_axon_ntff_profile_hook = lambda: _hook
    mod.set_axon_ntff_profile_hook = lambda h: None
    sys.modules["antenv.axon_hooks"] = mod

    from concourse import bass_utils

    bass_utils.upload_artifacts = lambda tmpdir: f"local:{tmpdir}"


def kernel(key, value, mem_W1, mem_b1, mem_W2, mem_b2, prec_W, prec_b,
           _trace=False, _tmpdir=None):
    global LAST_RESULTS
    key = np.asarray(key, dtype=np.float32)
    value = np.asarray(value, dtype=np.float32)
    mem_W1 = np.asarray(mem_W1, dtype=np.float32)
    mem_b1 = np.asarray(mem_b1, dtype=np.float32)
    mem_W2 = np.asarray(mem_W2, dtype=np.float32)
    mem_b2 = np.asarray(mem_b2, dtype=np.float32)
    prec_W = np.asarray(prec_W, dtype=np.float32)
    prec_b = np.asarray(prec_b, dtype=np.float32)

    if "nc" not in _CACHE:
        _CACHE["nc"] = build_nc(BC)
    nc = _CACHE["nc"]

    bf = ml_dtypes.bfloat16
    w1t = np.ascontiguousarray(mem_W1.T).astype(bf)          # [512, 64]
    b1 = np.ascontiguousarray(mem_b1[:, None])               # [64, 1]
    w2ta = np.ascontiguousarray(
        np.vstack([mem_W2.T, mem_b2[None, :]])).astype(bf)   # [65, 512]
    pwt = np.ascontiguousarray(prec_W.T).astype(bf)          # [512, 512]
    pb = np.ascontiguousarray(prec_b[None, :]).astype(bf)    # [1, 512]

    in_maps = []
    for c in range(N_CORES):
        sl = slice(c * BC, (c + 1) * BC)
        in_maps.append({
            "keyT": np.ascontiguousarray(key[sl].T).astype(bf),
            "value": np.ascontiguousarray(value[sl]).astype(bf),
            "w1t": w1t, "b1": b1, "w2ta": w2ta, "pwt": pwt, "pb": pb,
        })

    if _trace or os.environ.get("BASS_TRACE"):
        _install_prof_hook()
    res = run_bass_kernel_spmd(nc, in_maps, list(range(N_CORES)),
                               trace=_trace, tmpdir=_tmpdir)
    LAST_RESULTS = res

    F = np.concatenate([res.results[c]["F"] for c in range(N_CORES)], axis=0)
    pred = np.concatenate([res.results[c]["pred"] for c in range(N_CORES)], axis=0)
    prec = np.concatenate([res.results[c]["prec"] for c in range(N_CORES)], axis=0)
    err = np.concatenate([res.results[c]["err"] for c in range(N_CORES)], axis=0)
    return (F.astype(np.float32), pred.astype(np.float32),
            prec.astype(np.float32), err.astype(np.float32))
